# revision 63
# baseline (speedup 1.0000x reference)
"""BertSelfAttention Trainium2 kernel.

Full inputs in, full output out. Sharding: 8 cores = (batch b in {0,1}) x
(head-group hg in {0..3}); each core computes 4 heads of one batch and
produces the output feature slice out[b, :, hg*256:(hg+1)*256].

Per-core device program (all cores run the same NEFF, SPMD):
  xT [1024, 2048]      hidden_states[b].T, fp16
  QT/KT computed transposed [d, s] fp16; K^T lands in per-head
    ZERO-PADDED tiles (the head's 64 dims in their native rows, zeros in
    the other 64) so every scores matmul uses a full 128-row stationary
    with no tile_position -- FWL + the background weight buffer then hide
    the LDWEIGHTS (measured: scores MMs issue at N/2.4+2.5ns, ctx MMs at
    ~30ns -- both LDW-free)
  V computed [s, d] fp16, rows scaled by exp(mask), plus a per-head
    ones*exp(mask) column so the ctx matmul also yields softmax row sums
  scoresT [k, q]: per key-tile one [128, 128] stationary x [128, 512q]
    moving fp16 matmul per head, accumulated in 3-bank PSUM batches
  exp on ACT directly from PSUM (scale=1/8, bias=-4 folded in), fp16 out
  ctx[q, d] = expT.T @ [V|em] accumulated over 16 k-tiles, then
    per-partition normalize (batched reciprocal of the 4 row-sum
    columns) + V-bias add on DVE; one batched output DMA per 512 rows.

Engine budget (trace-measured): ACT exp = 17.2us/iteration (12
activations, the hard bound); PE = 13-16us/iteration.  The schedule is
therefore built to keep ACT 100% busy from the first exp on:

  * Emission interleaves a global work queue (ctx pieces of the previous
    iteration + projection fillers) BETWEEN the scores batches, ~1.8us
    of PE work per batch gap, so the PE absorbs its scores-psum WAR
    stalls doing useful work and the next batch's matmuls (and exps)
    always issue immediately -- the baseline's emit-all-batches-then-
    drip ordering cost a 3-4.5us ACT bubble at every iteration boundary.
  * Unconsumed queue work rolls across iteration boundaries (bounded by
    the e-tile triple buffering; the queue drains by iteration 3).
  * Iteration 7 runs all A-half batches first, then queues A's ctx
    between the B-half batches; only B's ctx remains after the last exp.

DMA: the critical startup set (wk + xt nb0, 1.5MB) is split between the
two HWDGE queues (SP and ACT) interleaved in ft order so the ft-serial
K projection starts on chunk 0; wq follows on both queues (ACT's last
issue lands before the first exp needs the ACT engine back); xt-nb1 +
nb2 follow on SP, nb3 + wv + constants on the Pool SWDGE.  Queue order
alone protects the critical window -- no dep-chaining, so iteration 0's
later batches (K-proj nb2/nb3-gated) are no longer starved by a
deferred-DMA trickle.

HAM: a short chain of dummy warm-up matmuls bridges from engine start
to the first DMA-fed projection.  No steady-state warmers: the
interleaved queue keeps PE gaps well under the 3.4us HAM idle window.

fp8 was evaluated and rejected: softmax weight concentration means
quantization errors do not average out (measured 7% output error for
fp8 q/k).  A custom-DVE polynomial exp offload validated numerically
but the custom-DVE lowering in this neuronxcc build fails in walrus
codegen ("ISA wrong length"), and standard-op DVE polynomials lose to
ACT (PSUM fp32 source caps DVE at 1x = 1.04ns/col vs ACT 0.83ns/col
before the ~6 passes needed).
"""

import numpy as np

B = 2
S = 2048
H = 1024
NH = 16
HD = 64

NCORES = 8
HPC = 4          # heads per core
DS = HPC * HD    # 256 output dims per core
FT = H // 128    # 8 f-tiles (contraction tiles for projections)
KT = S // 128    # 16 key tiles
ST = S // 128    # 16 s-tiles of V
QB = 4           # q blocks of 512
QBS = 512
VW = HPC * (HD + 1)  # 260: V columns + one em column per head

EXP_BIAS = -4.0  # uniform shift inside exp; cancels in softmax, guards fp16

WARMUP_MMS = 14       # dummy matmuls bridging engine start -> first proj
                      # (>3.4us of sustained activity so the HAM SHORT
                      # window fires and the projections run at 2.4GHz)
PUMP_BUDGET = 1.8     # us of queued PE work emitted after each batch
PUMP_BUDGET_SMALL = 0.7  # after the short (15,1) batch

_CACHE = {}


def _build_program(split_waits=True):
    import concourse.bass as bass
    import concourse.mybir as mybir
    import concourse.tile as tile
    from concourse.tile_rust import add_dep_helper
    from concourse.vector_clock import ScopedClock

    f32 = mybir.dt.float32
    f16 = mybir.dt.float16
    AF = mybir.ActivationFunctionType
    OP = mybir.AluOpType

    class SplitDrainTileContext(tile.TileContext):
        """The walrus build here rejects instructions with more than one
        sync wait ("Too many sync wait commands"); hoist excess waits onto
        preceding same-engine NOPs."""

        MAX_WAITS_PER_DRAIN = 1
        split_waits_enabled = True

        def _drain_and_barrier(self, tick_clock, wait_clock):
            drain_inst = self.nc.sync.drain()
            wait_clock.add_sem_waits(
                drain_inst.ins, ScopedClock({None: tick_clock.global_clock})
            )
            self.nc.all_engine_barrier()
            assert self.sems is not None
            popped = self.nc._tile_sem_poison_stack.pop()
            assert popped is self._sem_poison
            self.nc.clear_and_free_semaphores(list(self.sems.allocated().values()))
            self.nc.all_engine_barrier()
            if self.split_waits_enabled:
                self._split_multi_waits()

        def _split_multi_waits(self):
            k = self.MAX_WAITS_PER_DRAIN
            nc = self.nc
            for bb in nc.bb_map.values():
                il = bb.bb.instructions
                new = []
                for inst in il:
                    si = getattr(inst, "sync_info", None)
                    waits = list(si.on_wait) if si is not None and si.on_wait else []
                    if len(waits) > k:
                        for j in range(0, len(waits) - k, k):
                            nop = mybir.InstNoOp(
                                name=nc.get_next_instruction_name(),
                                engine=inst.engine,
                                sync_info=mybir.SyncInfo(
                                    on_wait=waits[j : j + k], on_update=[]
                                ),
                                bass_nofuse=True,
                            )
                            new.append(nop)
                        inst.sync_info = mybir.SyncInfo(
                            on_wait=waits[len(waits) - k :],
                            on_update=list(si.on_update) if si.on_update else [],
                        )
                    new.append(inst)
                il[:] = new

    nc = bass.Bass("TRN2", target_bir_lowering=False, debug=False,
                   num_devices=NCORES)

    # DRAM inputs.  xT stays [H, S]; the weights are repacked host-side so
    # each one is a single [128, FT*cols] transfer.
    xT_d = nc.dram_tensor("xT", [H, S], f16, kind="ExternalInput")
    wq_d = nc.dram_tensor("wq", [128, FT * DS], f16, kind="ExternalInput")
    wk_d = nc.dram_tensor("wk", [128, FT * DS], f16, kind="ExternalInput")
    wv_d = nc.dram_tensor("wv", [128, FT * VW], f16, kind="ExternalInput")
    bqk_d = nc.dram_tensor("bqk", [128, 4], f32, kind="ExternalInput")
    bvb_d = nc.dram_tensor("bvb", [128, DS], f32, kind="ExternalInput")
    em_d = nc.dram_tensor("em", [128, KT], f32, kind="ExternalInput")
    out_d = nc.dram_tensor("out", [S, DS], f32, kind="ExternalOutput")

    SplitDrainTileContext.split_waits_enabled = split_waits
    with SplitDrainTileContext(nc) as tc:
        from contextlib import ExitStack

        with ExitStack() as ctx:
            const = ctx.enter_context(tc.tile_pool(name="const", bufs=1))
            qk = ctx.enter_context(tc.tile_pool(name="qk", bufs=1))
            vp = ctx.enter_context(tc.tile_pool(name="vp", bufs=1))
            epool = ctx.enter_context(tc.tile_pool(name="epool", bufs=1))
            opool = ctx.enter_context(tc.tile_pool(name="opool", bufs=1))
            rpool = ctx.enter_context(tc.tile_pool(name="rpool", bufs=1))

            # ---- PE warm-up source (memset first so the warm-up matmuls
            # can start as soon as the engines come up) ----
            dummy = const.tile([128, 256], f16, tag="dummy", bufs=1,
                               name="dummy")
            nc.vector.memset(dummy[:], 0.0)

            # ---- constants (small, Pool-issued DMAs; bqk first -- the
            # kt_pad bias adds sit in the startup critical chain) ----
            bqk_sb = const.tile([128, 4], f32, tag="bqk", bufs=1, name="bqk_sb")
            nc.gpsimd.dma_start(bqk_sb[:], bqk_d.ap())
            bq_sb = [bqk_sb[:, m:m + 1] for m in range(2)]
            bk_sb = [bqk_sb[:, 2 + m:3 + m] for m in range(2)]
            # em/bvb tiles declared here; their DMAs are issued AFTER wq
            # in the Pool queue (em is first needed by the V-projections
            # at ~25us, bvb by the first eviction at ~45us -- issuing
            # them early would delay the critical wq transfer)
            em_sb = const.tile([128, KT], f32, tag="em", bufs=1, name="em_sb")
            bvb_sb = const.tile([128, DS], f32, tag="bvb", bufs=1, name="bvb_sb")
            ebias = const.tile([128, 1], f32, tag="ebias", bufs=1, name="ebias")
            nc.vector.memset(ebias[:], EXP_BIAS)
            # warm the ACT exp table while DMAs run
            warm = const.tile([128, 1], f32, tag="warm", bufs=1, name="warm")
            nc.scalar.activation(warm[:], ebias[:], AF.Exp)

            # ---- persistent activations ----
            # kt_pad[m][side]: K^T for head-pair m, one head per tile, the
            # head's 64 dims in their native partition rows and ZEROS in the
            # other 64 rows, so every scores matmul uses a full 128-row
            # stationary (FWL hides the LDWEIGHTS completely).
            qt = [qk.tile([128, S], f16, tag=f"qt{m}", bufs=1, name=f"qt{m}")
                  for m in range(2)]
            kt_pad = [[qk.tile([128, S], f16, tag=f"kt{m}{sd}", bufs=1,
                               name=f"kt{m}{sd}") for sd in range(2)]
                      for m in range(2)]
            # only the m0 zero-halves up front: the m1 pair (first read
            # by iteration 4's scores) is queued at iteration 1 so 3.5us
            # of DVE memset doesn't sit ahead of the startup-critical
            # K-projection drains in the in-order DVE queue
            nc.vector.memset(kt_pad[0][0][64:128, :], 0.0)
            nc.vector.memset(kt_pad[0][1][0:64, :], 0.0)
            vones = [vp.tile([128, VW], f16, tag=f"v{st}", bufs=1,
                             name=f"vones{st}") for st in range(ST)]

            # ---- input DMAs ----
            xw = ctx.enter_context(tc.tile_pool(name="xw", bufs=1))
            xt_all = xw.tile([128, FT * S], f16, tag="xt", bufs=1,
                             name="xt_all")

            def xt_ap(ft, c0, c1):
                return xt_all[:, ft * S + c0: ft * S + c1]

            wq_sb = xw.tile([128, FT * DS], f16, tag="wq", bufs=1, name="wq")
            wk_sb = xw.tile([128, FT * DS], f16, tag="wk", bufs=1, name="wk")
            wv_sb = xw.tile([128, FT * VW], f16, tag="wv", bufs=1, name="wv")

            def xt_dma(eng, ft, nb):
                fs = slice(ft * 128, (ft + 1) * 128)
                eng.dma_start(xt_ap(ft, nb * QBS, (nb + 1) * QBS),
                              xT_d.ap()[fs, nb * QBS:(nb + 1) * QBS])

            def xt_dma2(eng, ft, nb0_):
                # one 256KB transfer carrying TWO adjacent q-blocks of one
                # ft (contiguous in both DRAM and the packed xt layout)
                fs = slice(ft * 128, (ft + 1) * 128)
                eng.dma_start(xt_ap(ft, nb0_ * QBS, (nb0_ + 2) * QBS),
                              xT_d.ap()[fs, nb0_ * QBS:(nb0_ + 2) * QBS])

            # The startup fabric sustains ~320GB/s aggregate but only
            # ~100-160GB/s per queue, and queues share it fairly -- so
            # the critical set (wk + xt nb0 + wq, 2MB) must be the ONLY
            # traffic during the critical window.  Round-robin it across
            # all three queues in consumer order (the K projection is
            # ft-serial), non-critical bulk strictly behind per-queue:
            # nb1 split across the HWDGE queues (lands ~20us, for K-proj
            # m0 nb1), wv + nb2/nb3 pairs behind (lands ~25-28us, for
            # the V projections and K-proj nb2/nb3).  The ACT engine's
            # last issue stays ~3us clear of the first exp.
            def w_chunk(w_sb_, w_d_, c):
                cs = slice(c * 2 * DS, (c + 1) * 2 * DS)
                return (w_sb_[:, cs], w_d_.ap()[:, cs])

            # Critical K inputs (wk + xt nb0) on the two HWDGE queues,
            # interleaved in ft order for the ft-serial projection; wq as
            # two 256KB chunks on the Pool SWDGE right after bqk (lands
            # ~13us, before the Q projection can run anyway).
            for c in range(0, 4, 2):
                nc.sync.dma_start(*w_chunk(wk_sb, wk_d, c))
                nc.scalar.dma_start(*w_chunk(wk_sb, wk_d, c + 1))
                for f2 in range(2):
                    xt_dma(nc.sync, 2 * c + 2 * f2, 0)
                    xt_dma(nc.scalar, 2 * c + 2 * f2 + 1, 0)
            nc.gpsimd.dma_start(wq_sb[:, 0:4 * DS], wq_d.ap()[:, 0:4 * DS])
            nc.gpsimd.dma_start(wq_sb[:, 4 * DS:8 * DS],
                                wq_d.ap()[:, 4 * DS:8 * DS])

            # Everything past the critical set is released in two stages
            # gated on the K projection's progress: the DMA queues
            # process their entries in order but SHARE the fabric, so an
            # un-gated tail on one queue steals ~40% of the bandwidth
            # from another queue's still-critical transfers (measured:
            # xt nb0 slices landing at 22us instead of 15).  Stage 1
            # (xt nb1, for K-proj nb1 at ~23us) releases mid-K-chain;
            # stage 2 (nb2/nb3, wv, biases) only once nb1 is consumed.
            # First entry per queue per stage gets a semaphore dep on
            # the anchor; the rest follow in queue order (order-only
            # deps so the transfers still overlap each other).
            _dma_prev_q = {}

            def _gated(inst, anchor):
                eng = inst.ins.engine
                prev = _dma_prev_q.get(eng)
                add_dep_helper(inst.ins,
                               (anchor if prev is None else prev).ins,
                               sync=(prev is None), reason="dma-pacing")
                _dma_prev_q[eng] = inst

            def deferred_dmas_nb1(anchor):
                for ft in range(0, FT, 2):
                    fs = slice(ft * 128, (ft + 1) * 128)
                    _gated(nc.sync.dma_start(
                        xt_ap(ft, QBS, 2 * QBS),
                        xT_d.ap()[fs, QBS:2 * QBS]), anchor)
                    fs1 = slice((ft + 1) * 128, (ft + 2) * 128)
                    _gated(nc.scalar.dma_start(
                        xt_ap(ft + 1, QBS, 2 * QBS),
                        xT_d.ap()[fs1, QBS:2 * QBS]), anchor)

            def deferred_dmas_rest(anchor):
                # gpsimd's first entry anchors on `anchor`; Sync's
                # stage-2 entries just chain behind its stage-1 ones
                # (the per-queue DMA ring is FIFO, so they cannot start
                # before the nb1 transfers complete)
                _gated(nc.gpsimd.dma_start(em_sb[:], em_d.ap()), anchor)
                for ft in range(0, FT, 2):
                    _gated(nc.gpsimd.dma_start(
                        xt_ap(ft, 2 * QBS, 4 * QBS),
                        xT_d.ap()[slice(ft * 128, (ft + 1) * 128),
                                  2 * QBS:4 * QBS]), anchor)
                    _gated(nc.sync.dma_start(
                        xt_ap(ft + 1, 2 * QBS, 4 * QBS),
                        xT_d.ap()[slice((ft + 1) * 128, (ft + 2) * 128),
                                  2 * QBS:4 * QBS]), anchor)
                for vp2 in range(2):
                    cs = slice(vp2 * 4 * VW, (vp2 + 1) * 4 * VW)
                    _gated(nc.gpsimd.dma_start(
                        wv_sb[:, cs], wv_d.ap()[:, cs]), anchor)
                _gated(nc.gpsimd.dma_start(bvb_sb[:], bvb_d.ap()), anchor)

            def w_slice(w, ft, cols, c0, c1):
                return w[:, ft * cols + c0: ft * cols + c1]

            # ---- PSUM pools: proj 1 + scores 2x3 + ctx 1 = 8 banks ----
            ps_pj = ctx.enter_context(
                tc.tile_pool(name="ps_pj", bufs=1, space="PSUM"))
            ps_sc = ctx.enter_context(
                tc.tile_pool(name="ps_sc", bufs=2, space="PSUM"))
            ps_cx = ctx.enter_context(
                tc.tile_pool(name="ps_cx", bufs=1, space="PSUM"))

            mm = nc.tensor.matmul

            # ---- PE warm-up: dummy matmuls bridging engine start to the
            # first DMA-fed projection so the HAM clock-gate ramps to 8/8
            # before the dense work begins.  The chain writes the pj bank
            # (overwritten with start=True by the first projection).
            wup = ps_pj.tile([128, QBS], f32, tag="pj", name="warmup")
            prev_w = None
            for i in range(WARMUP_MMS):
                inst = mm(wup[:, 0:256], dummy[:, 0:128], dummy[:],
                          start=True, stop=True)
                if prev_w is not None:
                    add_dep_helper(inst.ins, prev_w.ins, sync=True,
                                   reason="warmup-chain")
                prev_w = inst

            # HAM bridge: unchained dummy matmuls into the (startup-idle)
            # ctx psum bank, interleaved between the DMA-gated projection
            # matmuls so the activity monitor never re-throttles during
            # the dribbled startup.  The ctx/v-proj allocations that
            # follow are WAR-ordered behind these writes automatically.
            bridge_ps = ps_cx.tile([128, 4 * (HD + 1)], f32, tag="cx",
                                   name="bridge")

            def bridge(n, dep=None):
                # dep-chained so the static scheduler places them right
                # after `dep` instead of hoisting all always-ready
                # bridges to the front
                for _ in range(n):
                    inst = mm(bridge_ps[:, 0:128], dummy[:, 0:128],
                              dummy[:, 0:128], start=True, stop=True)
                    if dep is not None:
                        add_dep_helper(inst.ins, dep.ins, sync=True,
                                       reason="ham-bridge")
                    dep = inst

            def warmer(dep):
                """One dummy matmul dep-chained on an exp: keeps the HAM
                activity window busy across the PE lulls of the thin
                (ACT-bound) late iterations, whose ~5us idle stretches
                otherwise re-throttle the PE clock to 1.2GHz."""
                ps = ps_pj.tile([128, QBS], f32, tag="pj", name="warmer")
                inst = mm(ps[:, 0:128], dummy[:, 0:128], dummy[:, 0:128],
                          start=True, stop=True)
                add_dep_helper(inst.ins, dep.ins, sync=True,
                               reason="ham-warmer")

            # iteration 4 carries the m1 projection pile -- its real PE
            # work keeps HAM warm, and warmers would displace it
            WARM_ITERS = {5, 6, 7}

            # ---- work units ----
            def qk_proj_block(w_sb, bias_ap, dst, m, nb, split_k=False,
                              pool=None, bridges=0, half=None):
                # half=0/1: project only 256 of the block's 512 positions
                # -- a queued piece that fits the ~1.8us batch-gap budget
                if half is None:
                    c0, cw = nb * QBS, QBS
                else:
                    c0, cw = nb * QBS + half * 256, 256
                ns = slice(c0, c0 + cw)
                if pool is None:
                    ps = ps_pj.tile([128, QBS], f32, tag="pj", name="pspj")
                else:
                    # startup only: borrow a scores-pool buffer so the Q
                    # projection does not WAR-wait the K drain on ps_pj
                    ps_full = ps_sc.tile([128, 3 * QBS], f32, tag="sc",
                                         name="pspjq")
                    ps = ps_full[:, 0:QBS]
                mms = []
                for ft in range(FT):
                    mms.append(mm(
                        ps[:, 0:cw],
                        w_slice(w_sb, ft, DS, m * 128, (m + 1) * 128),
                        xt_ap(ft, c0, c0 + cw),
                        start=(ft == 0), stop=(ft == FT - 1)))
                    if bridges and ft < FT - 1:
                        # keep the HAM activity window busy across the
                        # DMA-arrival-paced accumulation chain
                        bridge(bridges, dep=mms[-1])
                if split_k:
                    nc.vector.tensor_scalar_add(
                        kt_pad[m][0][0:64, ns], ps[0:64, 0:cw],
                        bias_ap[0:64, :])
                    nc.vector.tensor_scalar_add(
                        kt_pad[m][1][64:128, ns], ps[64:128, 0:cw],
                        bias_ap[64:128, :])
                else:
                    nc.vector.tensor_scalar_add(dst[:, ns], ps[:, 0:cw],
                                                bias_ap)
                return mms

            def v_proj_block(st):
                # alternate psum between the pj and (pre-ctx idle) cx
                # banks: consecutive V-projections then overlap the DVE
                # drain of one with the matmuls of the next instead of
                # WAR-serializing on a single bank
                ws0 = st * 128
                if st % 2 == 0:
                    ps = ps_pj.tile([128, QBS], f32, tag="pj", name="pspjv")
                else:
                    ps = ps_cx.tile([128, 4 * (HD + 1)], f32, tag="cx",
                                    name="pscxv")
                for ft in range(FT):
                    mm(ps[:, 0:VW],
                       xt_ap(ft, ws0, ws0 + 128),
                       w_slice(wv_sb, ft, VW, 0, VW),
                       start=(ft == 0), stop=(ft == FT - 1))
                nc.vector.tensor_scalar_mul(
                    vones[st][:], ps[:, 0:VW], em_sb[:, st:st + 1])
                for hh in range(HPC):
                    c = hh * (HD + 1) + HD
                    nc.gpsimd.tensor_copy(
                        vones[st][:, c:c + 1], em_sb[:, st:st + 1])

            BATCHES = [(0, 3), (3, 3), (6, 3), (9, 3), (12, 3), (15, 1)]

            def scores_batch(hp, qb, eA, eB, k0, nk, half=None):
                """Scores for `nk` key-tiles of 512 queries.  Stationary =
                the head's zero-padded K^T tile; moving = the full 128-row
                q tile (the other head's rows hit the zero weights)."""
                qs = slice(qb * QBS, (qb + 1) * QBS)
                w = nk * QBS
                es = slice(k0 * QBS, k0 * QBS + w)
                psA = psB = None
                if half is None or half == 0:
                    psA = ps_sc.tile([128, 3 * QBS], f32, tag="sc",
                                     name="pscA")
                if half is None or half == 1:
                    psB = ps_sc.tile([128, 3 * QBS], f32, tag="sc",
                                     name="pscB")
                for j in range(nk):
                    kt = k0 + j
                    ks0 = kt * 128
                    js = slice(j * QBS, (j + 1) * QBS)
                    for (ps, sd) in ((psA, 0), (psB, 1)):
                        if ps is None:
                            continue
                        mm(ps[:, js],
                           kt_pad[hp][sd][:, ks0:ks0 + 128],
                           qt[hp][:, qs])
                out = []
                for (ps, e) in ((psA, eA), (psB, eB)):
                    if ps is None:
                        continue
                    out.append(nc.scalar.activation(
                        e[:, es], ps[:, 0:w], AF.Exp, bias=ebias[:],
                        scale=0.125))
                return out

            def ctx_pieces(prev_state, fine_tail=False):
                """The ctx work for iteration `prev_state`, split into 10
                independently-emittable pieces (per head: 4 q-subtile
                matmul groups + 1 eviction), queued between the next
                iteration's scores batches.  With fine_tail (the very
                last head), each group evicts its own q-subtile as soon
                as its accumulation stops and the output ships in two
                halves, collapsing the end-of-kernel serial chain."""
                hp, qb, eA, eB = prev_state
                state = {"ot": None, "cps": {}}

                def mk_mm_group(a, e, qq):
                    def f():
                        cpsb = state["cps"].get(a)
                        if cpsb is None:
                            cpsb = ps_cx.tile([128, 4 * (HD + 1)], f32,
                                              tag="cx", name="cps")
                            state["cps"][a] = cpsb
                        hh = 2 * hp + a
                        cps = cpsb[:, qq * (HD + 1):(qq + 1) * (HD + 1)]
                        for ktile in range(KT):
                            lo = ktile * QBS + qq * 128
                            mm(cps, e[:, lo:lo + 128],
                               vones[ktile][:,
                                            hh * (HD + 1):(hh + 1) * (HD + 1)],
                               start=(ktile == 0), stop=(ktile == KT - 1))
                    return f

                def mk_evict(a):
                    def f():
                        hh = 2 * hp + a
                        cpsb = state["cps"][a]
                        # per-head output tile + DMA: head A's output
                        # ships while head B's ctx is still accumulating
                        ot = opool.tile([128, 4 * HD], f32, tag="ot",
                                        bufs=4, name="ot")
                        r4 = rpool.tile([128, 4], f32, tag="r", bufs=2,
                                        name="r")
                        nc.vector.reciprocal(
                            r4[:], cpsb[:, HD:4 * (HD + 1):HD + 1])
                        for qq in range(4):
                            cps = cpsb[:, qq * (HD + 1):(qq + 1) * (HD + 1)]
                            nc.vector.scalar_tensor_tensor(
                                ot[:, qq * HD:(qq + 1) * HD],
                                cps[:, 0:HD], r4[:, qq:qq + 1],
                                bvb_sb[:, hh * HD:(hh + 1) * HD],
                                op0=OP.mult, op1=OP.add)
                        qt0 = qb * 4 * 128
                        dram = out_d.ap()[qt0:qt0 + 4 * 128,
                                          hh * HD:(hh + 1) * HD]
                        nc.sync.dma_start(
                            dram.rearrange("(qq p) c -> p qq c", qq=4),
                            ot[:])
                    return f

                def mk_fine(a, e, qq):
                    def f():
                        mk_mm_group(a, e, qq)()
                        hh = 2 * hp + a
                        cpsb = state["cps"][a]
                        if qq == 0:
                            state["ot"] = opool.tile([128, 4 * HD], f32,
                                                     tag="ot", bufs=4,
                                                     name="ot")
                        ot = state["ot"]
                        r1 = rpool.tile([128, 4], f32, tag="r", bufs=2,
                                        name="r1")
                        nc.vector.reciprocal(
                            r1[:, 0:1],
                            cpsb[:, qq * (HD + 1) + HD:
                                 qq * (HD + 1) + HD + 1])
                        cps = cpsb[:, qq * (HD + 1):(qq + 1) * (HD + 1)]
                        nc.vector.scalar_tensor_tensor(
                            ot[:, qq * HD:(qq + 1) * HD],
                            cps[:, 0:HD], r1[:, 0:1],
                            bvb_sb[:, hh * HD:(hh + 1) * HD],
                            op0=OP.mult, op1=OP.add)
                        if qq in (1, 3):
                            p = qq // 2
                            qt0 = qb * 4 * 128 + p * 256
                            dram = out_d.ap()[qt0:qt0 + 256,
                                              hh * HD:(hh + 1) * HD]
                            nc.sync.dma_start(
                                dram.rearrange("(qq p) c -> p qq c", qq=2),
                                ot[:, p * 2 * HD:(p + 1) * 2 * HD])
                    return f

                pieces = []
                for a, e in ((0, eA), (1, eB)):
                    if fine_tail and a == 1:
                        for qq in range(4):
                            pieces.append((0.6, mk_fine(a, e, qq)))
                    else:
                        for qq in range(4):
                            pieces.append((0.5, mk_mm_group(a, e, qq)))
                        pieces.append((0.25, mk_evict(a)))
                return pieces

            # ---- emission schedule ----
            # A single global work queue of (pe_cost_us, fn, deadline)
            # pieces, pumped between scores batches so the PE fills its
            # psum-WAR windows with useful work and ACT never starves at
            # an iteration boundary.  Tile resolves data deps in EMISSION
            # order, so a projection emitted after its consumer (or a ctx
            # piece emitted after its e-tile buffer is reallocated)
            # silently reads the wrong data -- every piece therefore
            # carries a deadline key it*16+bi ("must be emitted before
            # batch bi of iteration it"; bi=15 means end of iteration)
            # and drain_due() force-emits due pieces, FIFO, before each
            # batch.  Front-heavy work (V/Q/K projections) thus rolls
            # into the back half's PE slack as far as its deadline
            # allows: the per-iteration PE/ACT balance is ~131us PE busy
            # against a 134us ACT span, but iterations 0-3 hold nearly
            # all the projections while 4-7 have ~5us slack each.
            from collections import deque
            work_q = deque()

            def key(it, bi=15):
                return it * 16 + bi

            def pump(budget):
                while work_q and budget > 0:
                    cost, fn, _ = work_q.popleft()
                    fn()
                    budget -= cost

            def drain_due(k):
                # force-emit, in FIFO order, everything up to and
                # including the LAST due piece (due pieces may sit
                # behind later-deadline ones)
                due = None
                for idx, (_, _, d) in enumerate(work_q):
                    if d <= k:
                        due = idx
                if due is not None:
                    for _ in range(due + 1):
                        _, fn, _ = work_q.popleft()
                        fn()

            def k1_block(nb, h=None):
                qk_proj_block(wk_sb, bk_sb[1], None, 1, nb, split_k=True,
                              half=h)

            def q1_block(nb, h=None):
                qk_proj_block(wq_sb, bq_sb[1], qt[1], 1, nb, half=h)

            PROJ_COST = 1.9   # 8 N=512 matmuls + pj-psum drain wait
            VPROJ_COST = 1.0  # 8 N=260 matmuls, drain overlapped (pj/cx)
            # fillers[it] = (cost, fn, deadline) queued at iteration it,
            # ordered by deadline.  V-projections must precede ctx(0)'s
            # pieces (FIFO gives that); the m0 q-block projections feed
            # iterations 2/3; the m1 K/Q projections feed specific
            # batches of iteration 4 (kt_pad[1] keys nb*512.. are first
            # read by the batch covering them), so they may slide into
            # iteration 4 itself.
            fillers = {
                0: [(PROJ_COST, lambda: qk_proj_block(
                        wq_sb, bq_sb[0], qt[0], 0, 2), key(2, 0))]
                   + [(VPROJ_COST, (lambda st=st: v_proj_block(st)),
                       key(2, 15))
                      for st in range(6)],
                1: [(0.1, lambda: nc.vector.memset(
                        kt_pad[1][0][64:128, :], 0.0), key(2)),
                    (0.1, lambda: nc.vector.memset(
                        kt_pad[1][1][0:64, :], 0.0), key(2))]
                   + [(VPROJ_COST, (lambda st=st: v_proj_block(st)),
                       key(2, 15))
                      for st in range(6, 12)]
                   + [(PROJ_COST, lambda: qk_proj_block(
                          wq_sb, bq_sb[0], qt[0], 0, 3), key(3, 0))],
                2: [(VPROJ_COST, (lambda st=st: v_proj_block(st)),
                     key(2, 15))
                    for st in range(12, ST)],
                # m1 projections as 256-position HALF blocks (~0.95us
                # each, fits the batch-gap budget) with deadlines at the
                # exact batch that first reads each half's keys
                3: [(1.0, lambda: k1_block(0, 0), key(3, 2)),
                    (1.0, lambda: k1_block(0, 1), key(3, 3)),
                    (1.0, lambda: q1_block(0, 0), key(3, 4)),
                    (1.0, lambda: q1_block(0, 1), key(3, 5)),
                    (1.0, lambda: k1_block(1, 0), key(4, 1)),
                    (1.0, lambda: k1_block(1, 1), key(4, 2))],
                4: [(1.0, lambda: k1_block(2, 0), key(4, 2)),
                    (1.0, lambda: k1_block(2, 1), key(4, 3)),
                    (1.0, lambda: k1_block(3, 0), key(4, 4)),
                    (1.0, lambda: k1_block(3, 1), key(4, 4)),
                    (1.0, lambda: q1_block(1, 0), key(5, 0)),
                    (1.0, lambda: q1_block(1, 1), key(5, 0))],
                5: [(1.0, lambda: q1_block(2, 0), key(6, 0)),
                    (1.0, lambda: q1_block(2, 1), key(6, 0))],
                6: [(1.0, lambda: q1_block(3, 0), key(7, 0)),
                    (1.0, lambda: q1_block(3, 1), key(7, 0))],
            }

            def queue_ctx(state, j):
                # ctx(j)'s A-half must be emitted before iteration j+3
                # reallocates eA's buffer (bufs=3); eB has bufs=4, so the
                # B-half may lag one iteration further into the back
                # half's PE slack
                pieces = ctx_pieces(state)
                work_q.extend((c, f, key(j + 2)) for (c, f) in pieces[:5])
                work_q.extend((c, f, key(j + 3)) for (c, f) in pieces[5:])

            prev = None
            for it in range(8):
                hp, qb = divmod(it, QB)
                eA = epool.tile([128, KT * QBS], f16, tag="eA", bufs=3,
                                name="eA")
                eB = epool.tile([128, KT * QBS], f16, tag="eB", bufs=4,
                                name="eB")
                # queue this iteration's fillers, then pending ctx
                # pieces.  ctx(0) is queued at iteration 2, AFTER the
                # last V-projections (its matmul groups read all 16
                # vones; FIFO order guarantees the precedence); ctx(j>0)
                # is queued at iteration j+1.
                work_q.extend(fillers.get(it, []))
                if it == 2:
                    queue_ctx(state0, 0)
                if prev is not None and it >= 2:
                    queue_ctx(prev, it - 1)

                def batch(k0, nk, half=None):
                    es = scores_batch(hp, qb, eA, eB, k0, nk, half)
                    if it in WARM_ITERS:
                        for e_inst in es:
                            warmer(e_inst)

                if it == 0:
                    # m0 Q/K projections interleaved with iteration-0's
                    # scores batches, each batch emitted as soon as the K
                    # key-blocks it reads are projected; HAM bridge
                    # matmuls (dep-chained per projection matmul) fill
                    # the DMA-arrival-paced stretches so the PE
                    # clock-gate stays at 8/8 through the startup.
                    # each K-projection emitted one batch AHEAD of its
                    # first consumer: the psum-drain adds' PE waits
                    # resolve coarsely (they fire only after later
                    # scheduled matmuls), so the extra batch of slack
                    # keeps them off the exp critical path
                    k_mms = qk_proj_block(wk_sb, bk_sb[0], None, 0, 0,
                                          split_k=True, bridges=2)
                    qk_proj_block(wq_sb, bq_sb[0], qt[0], 0, 0, pool="sc")
                    deferred_dmas_nb1(k_mms[3])
                    batch(0, 3)
                    k1_mms = qk_proj_block(wk_sb, bk_sb[0], None, 0, 1,
                                           split_k=True)
                    deferred_dmas_rest(k1_mms[-1])
                    qk_proj_block(wk_sb, bk_sb[0], None, 0, 2, split_k=True,
                                  bridges=1)
                    batch(3, 3)
                    qk_proj_block(wk_sb, bk_sb[0], None, 0, 3, split_k=True,
                                  bridges=1)
                    batch(6, 3)
                    qk_proj_block(wq_sb, bq_sb[0], qt[0], 0, 1)
                    batch(9, 3)
                    batch(12, 3)
                    pump(PROJ_COST)
                    batch(15, 1)
                    pump(2 * VPROJ_COST)
                elif it == 7:
                    # last iteration: all A-half batches first (draining
                    # the remaining queue); A's full ctx+eviction runs
                    # during the first B exp's drain; B's ctx then
                    # accumulates BATCH-SYNCHRONOUSLY (each batch's
                    # key-tiles right after its exp), so after the final
                    # exp only 4 matmuls + the per-q-subtile eviction
                    # and two half-output DMAs remain.
                    for bi, (k0, nk) in enumerate(BATCHES):
                        drain_due(key(it, bi))
                        batch(k0, nk, half=0)
                        pump(PUMP_BUDGET if nk == 3 else PUMP_BUDGET_SMALL)
                    # drain ALL queue leftovers (ctx(6) etc.) before any
                    # B-half ctx touches the shared cx psum bank -- a
                    # late ctx(6) piece emitted mid-bsync would clobber
                    # the in-progress B accumulation (bufs=1)
                    pump(1e9)
                    pieces7 = ctx_pieces((hp, qb, eA, eB))
                    a_pieces = pieces7[:5]

                    bst = {}

                    def bsync(k0, nk):
                        # partial ctx accumulation for head B over this
                        # batch's key-tiles, all 4 q-subtiles.  start=True
                        # clears the WHOLE bank's has_written bits, so it
                        # is issued exactly once (first matmul of the
                        # bank); each region's first write then lands on
                        # has_written=0 and overwrites, later ones
                        # accumulate -- interleaved per-region chains
                        # with per-region start flags corrupt each other.
                        if "cps" not in bst:
                            bst["cps"] = ps_cx.tile([128, 4 * (HD + 1)],
                                                    f32, tag="cx",
                                                    name="cpsB7")
                        cpsb = bst["cps"]
                        for qq in range(4):
                            cps = cpsb[:, qq * (HD + 1):(qq + 1) * (HD + 1)]
                            for j in range(nk):
                                kt = k0 + j
                                lo = kt * QBS + qq * 128
                                mm(cps, eB[:, lo:lo + 128],
                                   vones[kt][:, (2 * hp + 1) * (HD + 1):
                                             (2 * hp + 2) * (HD + 1)],
                                   start=(kt == 0 and qq == 0),
                                   stop=(kt == KT - 1))

                    # B-half gaps hold only ~1.6us of ACT drain each (one
                    # exp per batch), so A's 2.8us of ctx splits across
                    # the first two gaps and the first two bsyncs share
                    # gap 3
                    after_b = {
                        0: [a_pieces[0][1], a_pieces[1][1]],
                        1: [a_pieces[2][1], a_pieces[3][1], a_pieces[4][1]],
                        2: [lambda: bsync(0, 3), lambda: bsync(3, 3)],
                        3: [lambda: bsync(6, 3)],
                        4: [lambda: bsync(9, 3)],
                        5: [lambda: bsync(12, 3)],
                    }
                    for bi, (k0, nk) in enumerate(BATCHES):
                        batch(k0, nk, half=1)
                        for f in after_b[bi]:
                            f()
                    bsync(15, 1)
                    # B eviction: all 4 regions stop together in
                    # bsync(15,1), so one batched reciprocal + 4
                    # normalizes, output in two halves
                    cpsb = bst["cps"]
                    hh = 2 * hp + 1
                    otB = opool.tile([128, 4 * HD], f32, tag="ot", bufs=4,
                                     name="otB7")
                    r4 = rpool.tile([128, 4], f32, tag="r", bufs=2,
                                    name="r4B7")
                    nc.vector.reciprocal(
                        r4[:], cpsb[:, HD:4 * (HD + 1):HD + 1])
                    for qq in range(4):
                        cps = cpsb[:, qq * (HD + 1):(qq + 1) * (HD + 1)]
                        nc.vector.scalar_tensor_tensor(
                            otB[:, qq * HD:(qq + 1) * HD],
                            cps[:, 0:HD], r4[:, qq:qq + 1],
                            bvb_sb[:, hh * HD:(hh + 1) * HD],
                            op0=OP.mult, op1=OP.add)
                        if qq == 1:
                            qt0 = qb * 4 * 128
                            dram = out_d.ap()[qt0:qt0 + 256,
                                              hh * HD:(hh + 1) * HD]
                            nc.sync.dma_start(
                                dram.rearrange("(qq p) c -> p qq c", qq=2),
                                otB[:, 0:2 * HD])
                        elif qq >= 2:
                            # last two q-subtiles ship individually so
                            # the final transfer is only 32KB
                            qt0 = qb * 4 * 128 + qq * 128
                            dram = out_d.ap()[qt0:qt0 + 128,
                                              hh * HD:(hh + 1) * HD]
                            nc.sync.dma_start(
                                dram, otB[:, qq * HD:(qq + 1) * HD])
                    prev = None
                    continue
                else:
                    for bi, (k0, nk) in enumerate(BATCHES):
                        drain_due(key(it, bi))
                        batch(k0, nk)
                        pump(PUMP_BUDGET if nk == 3 else PUMP_BUDGET_SMALL)
                drain_due(key(it))
                if it == 0:
                    state0 = (hp, qb, eA, eB)
                prev = (hp, qb, eA, eB)

    return nc


def _get_program(split_waits=True):
    key = ("nc", split_waits)
    if key not in _CACHE:
        _CACHE[key] = _build_program(split_waits)
    return _CACHE[key]


def _make_in_maps(hidden_states, attention_mask, Wq, bq, Wk, bk, Wv, bv):
    hidden = np.ascontiguousarray(np.asarray(hidden_states, dtype=np.float32))
    mask = np.asarray(attention_mask, dtype=np.float32)
    Wq = np.asarray(Wq, dtype=np.float32)
    Wk = np.asarray(Wk, dtype=np.float32)
    Wv = np.asarray(Wv, dtype=np.float32)
    bq = np.asarray(bq, dtype=np.float32)
    bk = np.asarray(bk, dtype=np.float32)
    bv = np.asarray(bv, dtype=np.float32)

    WqT = Wq.T  # [in, out]
    WkT = Wk.T
    WvT = Wv.T

    def pack_ft(w):  # [H, C] -> [128, FT*C] with col block ft*C
        C = w.shape[1]
        out = np.empty((128, FT * C), np.float16)
        for ft in range(FT):
            out[:, ft * C:(ft + 1) * C] = w[ft * 128:(ft + 1) * 128, :]
        return np.ascontiguousarray(out)

    in_maps = []
    for c in range(NCORES):
        b, hg = divmod(c, HPC)
        cols = slice(hg * DS, (hg + 1) * DS)
        xT = np.ascontiguousarray(hidden[b].T.astype(np.float16))
        wq = pack_ft(WqT[:, cols].astype(np.float16))
        wk = pack_ft(WkT[:, cols].astype(np.float16))
        wv_base = WvT[:, cols]
        wvT = np.zeros((H, VW), np.float32)
        for hh in range(HPC):
            wvT[:, hh * (HD + 1):hh * (HD + 1) + HD] = \
                wv_base[:, hh * HD:(hh + 1) * HD]
        wv = pack_ft(wvT.astype(np.float16))
        bqk = np.empty((128, 4), np.float32)
        bqk[:, 0] = bq[cols][0:128]
        bqk[:, 1] = bq[cols][128:256]
        bqk[:, 2] = bk[cols][0:128]
        bqk[:, 3] = bk[cols][128:256]
        bvb = np.ascontiguousarray(np.tile(bv[cols][None, :], (128, 1)))
        em = np.ascontiguousarray(
            np.exp(mask[b, 0, 0, :]).reshape(KT, 128).T.astype(np.float32))
        in_maps.append({
            "xT": xT, "wq": wq, "wk": wk, "wv": wv,
            "bqk": np.ascontiguousarray(bqk), "bvb": bvb, "em": em,
        })
    return in_maps


def _assemble(results):
    out = np.empty((B, S, H), np.float32)
    for c in range(NCORES):
        b, hg = divmod(c, HPC)
        out[b][:, hg * DS:(hg + 1) * DS] = results[c]["out"]
    return out


def _run(in_maps, trace=False):
    from concourse.bass_utils import run_bass_kernel_spmd
    nc = _get_program()
    return run_bass_kernel_spmd(
        nc, in_maps, core_ids=list(range(NCORES)), trace=trace)


def kernel(**inputs):
    in_maps = _make_in_maps(**inputs)
    res = _run(in_maps, trace=False)
    return _assemble(res.results)


# revision 65
# speedup vs baseline: 1.0110x; 1.0110x over previous
"""BertSelfAttention Trainium2 kernel.

Full inputs in, full output out. Sharding: 8 cores = (batch b in {0,1}) x
(head-group hg in {0..3}); each core computes 4 heads of one batch and
produces the output feature slice out[b, :, hg*256:(hg+1)*256].

Per-core device program (all cores run the same NEFF, SPMD):
  xT [1024, 2048]      hidden_states[b].T, fp16
  QT/KT computed transposed [d, s] fp16; K^T lands in per-head
    ZERO-PADDED tiles (the head's 64 dims in their native rows, zeros in
    the other 64) so every scores matmul uses a full 128-row stationary
    with no tile_position -- FWL + the background weight buffer then hide
    the LDWEIGHTS (measured: scores MMs issue at N/2.4+2.5ns, ctx MMs at
    ~30ns -- both LDW-free)
  V computed [s, d] fp16, rows scaled by exp(mask), plus a per-head
    ones*exp(mask) column so the ctx matmul also yields softmax row sums
  scoresT [k, q]: per key-tile one [128, 128] stationary x [128, 512q]
    moving fp16 matmul per head, accumulated in 3-bank PSUM batches
  exp on ACT directly from PSUM (scale=1/8, bias=-4 folded in), fp16 out
  ctx[q, d] = expT.T @ [V|em] accumulated over 16 k-tiles, then
    per-partition normalize (batched reciprocal of the 4 row-sum
    columns) + V-bias add on DVE; one batched output DMA per 512 rows.

Engine budget (trace-measured): ACT exp = 17.2us/iteration (12
activations, the hard bound); PE = 13-16us/iteration.  The schedule is
therefore built to keep ACT 100% busy from the first exp on:

  * Emission interleaves a global work queue (ctx pieces of the previous
    iteration + projection fillers) BETWEEN the scores batches, ~1.8us
    of PE work per batch gap, so the PE absorbs its scores-psum WAR
    stalls doing useful work and the next batch's matmuls (and exps)
    always issue immediately -- the baseline's emit-all-batches-then-
    drip ordering cost a 3-4.5us ACT bubble at every iteration boundary.
  * Unconsumed queue work rolls across iteration boundaries (bounded by
    the e-tile triple buffering; the queue drains by iteration 3).
  * Iteration 7 runs all A-half batches first, then queues A's ctx
    between the B-half batches; only B's ctx remains after the last exp.

DMA: the critical startup set (wk + xt nb0, 1.5MB) is split between the
two HWDGE queues (SP and ACT) interleaved in ft order so the ft-serial
K projection starts on chunk 0; wq follows on both queues (ACT's last
issue lands before the first exp needs the ACT engine back); xt-nb1 +
nb2 follow on SP, nb3 + wv + constants on the Pool SWDGE.  Queue order
alone protects the critical window -- no dep-chaining, so iteration 0's
later batches (K-proj nb2/nb3-gated) are no longer starved by a
deferred-DMA trickle.

HAM: a short chain of dummy warm-up matmuls bridges from engine start
to the first DMA-fed projection.  No steady-state warmers: the
interleaved queue keeps PE gaps well under the 3.4us HAM idle window.

fp8 was evaluated and rejected: softmax weight concentration means
quantization errors do not average out (measured 7% output error for
fp8 q/k).  A custom-DVE polynomial exp offload validated numerically
but the custom-DVE lowering in this neuronxcc build fails in walrus
codegen ("ISA wrong length"), and standard-op DVE polynomials lose to
ACT (PSUM fp32 source caps DVE at 1x = 1.04ns/col vs ACT 0.83ns/col
before the ~6 passes needed).
"""

import numpy as np

B = 2
S = 2048
H = 1024
NH = 16
HD = 64

NCORES = 8
HPC = 4          # heads per core
DS = HPC * HD    # 256 output dims per core
FT = H // 128    # 8 f-tiles (contraction tiles for projections)
KT = S // 128    # 16 key tiles
ST = S // 128    # 16 s-tiles of V
QB = 4           # q blocks of 512
QBS = 512
VW = HPC * (HD + 1)  # 260: V columns + one em column per head

EXP_BIAS = -4.0  # uniform shift inside exp; cancels in softmax, guards fp16

WARMUP_MMS = 14       # dummy matmuls bridging engine start -> first proj
                      # (>3.4us of sustained activity so the HAM SHORT
                      # window fires and the projections run at 2.4GHz)
PUMP_BUDGET = 1.8     # us of queued PE work emitted after each batch
PUMP_BUDGET_SMALL = 0.7  # after the short (15,1) batch

_CACHE = {}


def _build_program(split_waits=True):
    import concourse.bass as bass
    import concourse.mybir as mybir
    import concourse.tile as tile
    from concourse.tile_rust import add_dep_helper
    from concourse.vector_clock import ScopedClock

    f32 = mybir.dt.float32
    f16 = mybir.dt.float16
    AF = mybir.ActivationFunctionType
    OP = mybir.AluOpType

    class SplitDrainTileContext(tile.TileContext):
        """The walrus build here rejects instructions with more than one
        sync wait ("Too many sync wait commands"); hoist excess waits onto
        preceding same-engine NOPs."""

        MAX_WAITS_PER_DRAIN = 1
        split_waits_enabled = True

        def _drain_and_barrier(self, tick_clock, wait_clock):
            drain_inst = self.nc.sync.drain()
            wait_clock.add_sem_waits(
                drain_inst.ins, ScopedClock({None: tick_clock.global_clock})
            )
            self.nc.all_engine_barrier()
            assert self.sems is not None
            popped = self.nc._tile_sem_poison_stack.pop()
            assert popped is self._sem_poison
            self.nc.clear_and_free_semaphores(list(self.sems.allocated().values()))
            self.nc.all_engine_barrier()
            if self.split_waits_enabled:
                self._split_multi_waits()

        def _split_multi_waits(self):
            k = self.MAX_WAITS_PER_DRAIN
            nc = self.nc
            for bb in nc.bb_map.values():
                il = bb.bb.instructions
                new = []
                for inst in il:
                    si = getattr(inst, "sync_info", None)
                    waits = list(si.on_wait) if si is not None and si.on_wait else []
                    if len(waits) > k:
                        for j in range(0, len(waits) - k, k):
                            nop = mybir.InstNoOp(
                                name=nc.get_next_instruction_name(),
                                engine=inst.engine,
                                sync_info=mybir.SyncInfo(
                                    on_wait=waits[j : j + k], on_update=[]
                                ),
                                bass_nofuse=True,
                            )
                            new.append(nop)
                        inst.sync_info = mybir.SyncInfo(
                            on_wait=waits[len(waits) - k :],
                            on_update=list(si.on_update) if si.on_update else [],
                        )
                    new.append(inst)
                il[:] = new

    nc = bass.Bass("TRN2", target_bir_lowering=False, debug=False,
                   num_devices=NCORES)

    # DRAM inputs.  xT stays [H, S]; the weights are repacked host-side so
    # each one is a single [128, FT*cols] transfer.
    xT_d = nc.dram_tensor("xT", [H, S], f16, kind="ExternalInput")
    wq_d = nc.dram_tensor("wq", [128, FT * DS], f16, kind="ExternalInput")
    wk_d = nc.dram_tensor("wk", [128, FT * DS], f16, kind="ExternalInput")
    wv_d = nc.dram_tensor("wv", [128, FT * VW], f16, kind="ExternalInput")
    bqk_d = nc.dram_tensor("bqk", [128, 4], f32, kind="ExternalInput")
    bvb_d = nc.dram_tensor("bvb", [128, DS], f32, kind="ExternalInput")
    em_d = nc.dram_tensor("em", [128, KT], f32, kind="ExternalInput")
    out_d = nc.dram_tensor("out", [S, DS], f32, kind="ExternalOutput")

    SplitDrainTileContext.split_waits_enabled = split_waits
    with SplitDrainTileContext(nc) as tc:
        from contextlib import ExitStack

        with ExitStack() as ctx:
            const = ctx.enter_context(tc.tile_pool(name="const", bufs=1))
            qk = ctx.enter_context(tc.tile_pool(name="qk", bufs=1))
            vp = ctx.enter_context(tc.tile_pool(name="vp", bufs=1))
            epool = ctx.enter_context(tc.tile_pool(name="epool", bufs=1))
            opool = ctx.enter_context(tc.tile_pool(name="opool", bufs=1))
            rpool = ctx.enter_context(tc.tile_pool(name="rpool", bufs=1))

            # ---- PE warm-up source (memset first so the warm-up matmuls
            # can start as soon as the engines come up) ----
            dummy = const.tile([128, 256], f16, tag="dummy", bufs=1,
                               name="dummy")
            nc.vector.memset(dummy[:], 0.0)

            # ---- constants (small, Pool-issued DMAs; bqk first -- the
            # kt_pad bias adds sit in the startup critical chain) ----
            bqk_sb = const.tile([128, 4], f32, tag="bqk", bufs=1, name="bqk_sb")
            nc.gpsimd.dma_start(bqk_sb[:], bqk_d.ap())
            bq_sb = [bqk_sb[:, m:m + 1] for m in range(2)]
            bk_sb = [bqk_sb[:, 2 + m:3 + m] for m in range(2)]
            # em/bvb tiles declared here; their DMAs are issued AFTER wq
            # in the Pool queue (em is first needed by the V-projections
            # at ~25us, bvb by the first eviction at ~45us -- issuing
            # them early would delay the critical wq transfer)
            em_sb = const.tile([128, KT], f32, tag="em", bufs=1, name="em_sb")
            bvb_sb = const.tile([128, DS], f32, tag="bvb", bufs=1, name="bvb_sb")
            ebias = const.tile([128, 1], f32, tag="ebias", bufs=1, name="ebias")
            nc.vector.memset(ebias[:], EXP_BIAS)
            # warm the ACT exp table while DMAs run
            warm = const.tile([128, 1], f32, tag="warm", bufs=1, name="warm")
            nc.scalar.activation(warm[:], ebias[:], AF.Exp)

            # ---- persistent activations ----
            # kt_pad[m][side]: K^T for head-pair m, one head per tile, the
            # head's 64 dims in their native partition rows and ZEROS in the
            # other 64 rows, so every scores matmul uses a full 128-row
            # stationary (FWL hides the LDWEIGHTS completely).
            qt = [qk.tile([128, S], f16, tag=f"qt{m}", bufs=1, name=f"qt{m}")
                  for m in range(2)]
            kt_pad = [[qk.tile([128, S], f16, tag=f"kt{m}{sd}", bufs=1,
                               name=f"kt{m}{sd}") for sd in range(2)]
                      for m in range(2)]
            # only the m0 zero-halves up front: the m1 pair (first read
            # by iteration 4's scores) is queued at iteration 1 so 3.5us
            # of DVE memset doesn't sit ahead of the startup-critical
            # K-projection drains in the in-order DVE queue
            nc.vector.memset(kt_pad[0][0][64:128, :], 0.0)
            nc.vector.memset(kt_pad[0][1][0:64, :], 0.0)
            vones = [vp.tile([128, VW], f16, tag=f"v{st}", bufs=1,
                             name=f"vones{st}") for st in range(ST)]

            # ---- input DMAs ----
            xw = ctx.enter_context(tc.tile_pool(name="xw", bufs=1))
            xt_all = xw.tile([128, FT * S], f16, tag="xt", bufs=1,
                             name="xt_all")

            def xt_ap(ft, c0, c1):
                return xt_all[:, ft * S + c0: ft * S + c1]

            wq_sb = xw.tile([128, FT * DS], f16, tag="wq", bufs=1, name="wq")
            wk_sb = xw.tile([128, FT * DS], f16, tag="wk", bufs=1, name="wk")
            wv_sb = xw.tile([128, FT * VW], f16, tag="wv", bufs=1, name="wv")

            def xt_dma(eng, ft, nb):
                fs = slice(ft * 128, (ft + 1) * 128)
                eng.dma_start(xt_ap(ft, nb * QBS, (nb + 1) * QBS),
                              xT_d.ap()[fs, nb * QBS:(nb + 1) * QBS])

            def xt_dma2(eng, ft, nb0_):
                # one 256KB transfer carrying TWO adjacent q-blocks of one
                # ft (contiguous in both DRAM and the packed xt layout)
                fs = slice(ft * 128, (ft + 1) * 128)
                eng.dma_start(xt_ap(ft, nb0_ * QBS, (nb0_ + 2) * QBS),
                              xT_d.ap()[fs, nb0_ * QBS:(nb0_ + 2) * QBS])

            # The startup fabric sustains ~320GB/s aggregate but only
            # ~100-160GB/s per queue, and queues share it fairly -- so
            # the critical set (wk + xt nb0 + wq, 2MB) must be the ONLY
            # traffic during the critical window.  Round-robin it across
            # all three queues in consumer order (the K projection is
            # ft-serial), non-critical bulk strictly behind per-queue:
            # nb1 split across the HWDGE queues (lands ~20us, for K-proj
            # m0 nb1), wv + nb2/nb3 pairs behind (lands ~25-28us, for
            # the V projections and K-proj nb2/nb3).  The ACT engine's
            # last issue stays ~3us clear of the first exp.
            def w_chunk(w_sb_, w_d_, c):
                cs = slice(c * 2 * DS, (c + 1) * 2 * DS)
                return (w_sb_[:, cs], w_d_.ap()[:, cs])

            # Critical K inputs (wk + xt nb0) on the two HWDGE queues,
            # interleaved in ft order for the ft-serial projection; wq as
            # two 256KB chunks on the Pool SWDGE right after bqk (lands
            # ~13us, before the Q projection can run anyway).
            for c in range(0, 4, 2):
                nc.sync.dma_start(*w_chunk(wk_sb, wk_d, c))
                nc.scalar.dma_start(*w_chunk(wk_sb, wk_d, c + 1))
                for f2 in range(2):
                    xt_dma(nc.sync, 2 * c + 2 * f2, 0)
                    xt_dma(nc.scalar, 2 * c + 2 * f2 + 1, 0)
            nc.gpsimd.dma_start(wq_sb[:, 0:4 * DS], wq_d.ap()[:, 0:4 * DS])
            nc.gpsimd.dma_start(wq_sb[:, 4 * DS:8 * DS],
                                wq_d.ap()[:, 4 * DS:8 * DS])

            # Everything past the critical set is released in two stages
            # gated on the K projection's progress: the DMA queues
            # process their entries in order but SHARE the fabric, so an
            # un-gated tail on one queue steals ~40% of the bandwidth
            # from another queue's still-critical transfers (measured:
            # xt nb0 slices landing at 22us instead of 15).  Stage 1
            # (xt nb1, for K-proj nb1 at ~23us) releases mid-K-chain;
            # stage 2 (nb2/nb3, wv, biases) only once nb1 is consumed.
            # First entry per queue per stage gets a semaphore dep on
            # the anchor; the rest follow in queue order (order-only
            # deps so the transfers still overlap each other).
            _dma_prev_q = {}

            def _gated(inst, anchor):
                eng = inst.ins.engine
                prev = _dma_prev_q.get(eng)
                add_dep_helper(inst.ins,
                               (anchor if prev is None else prev).ins,
                               sync=(prev is None), reason="dma-pacing")
                _dma_prev_q[eng] = inst

            def deferred_dmas_nb1(anchor):
                for ft in range(0, FT, 2):
                    fs = slice(ft * 128, (ft + 1) * 128)
                    _gated(nc.sync.dma_start(
                        xt_ap(ft, QBS, 2 * QBS),
                        xT_d.ap()[fs, QBS:2 * QBS]), anchor)
                    fs1 = slice((ft + 1) * 128, (ft + 2) * 128)
                    _gated(nc.scalar.dma_start(
                        xt_ap(ft + 1, QBS, 2 * QBS),
                        xT_d.ap()[fs1, QBS:2 * QBS]), anchor)

            def deferred_dmas_rest(anchor):
                # gpsimd's first entry anchors on `anchor`; Sync's
                # stage-2 entries just chain behind its stage-1 ones
                # (the per-queue DMA ring is FIFO, so they cannot start
                # before the nb1 transfers complete)
                _gated(nc.gpsimd.dma_start(em_sb[:], em_d.ap()), anchor)
                for ft in range(0, FT, 2):
                    _gated(nc.gpsimd.dma_start(
                        xt_ap(ft, 2 * QBS, 4 * QBS),
                        xT_d.ap()[slice(ft * 128, (ft + 1) * 128),
                                  2 * QBS:4 * QBS]), anchor)
                    _gated(nc.sync.dma_start(
                        xt_ap(ft + 1, 2 * QBS, 4 * QBS),
                        xT_d.ap()[slice((ft + 1) * 128, (ft + 2) * 128),
                                  2 * QBS:4 * QBS]), anchor)
                for vp2 in range(2):
                    cs = slice(vp2 * 4 * VW, (vp2 + 1) * 4 * VW)
                    _gated(nc.gpsimd.dma_start(
                        wv_sb[:, cs], wv_d.ap()[:, cs]), anchor)
                _gated(nc.gpsimd.dma_start(bvb_sb[:], bvb_d.ap()), anchor)

            def w_slice(w, ft, cols, c0, c1):
                return w[:, ft * cols + c0: ft * cols + c1]

            # ---- PSUM pools: proj 1 + scores 2x3 + ctx 1 = 8 banks ----
            ps_pj = ctx.enter_context(
                tc.tile_pool(name="ps_pj", bufs=1, space="PSUM"))
            ps_sc = ctx.enter_context(
                tc.tile_pool(name="ps_sc", bufs=2, space="PSUM"))
            ps_cx = ctx.enter_context(
                tc.tile_pool(name="ps_cx", bufs=1, space="PSUM"))

            mm = nc.tensor.matmul

            # ---- PE warm-up: dummy matmuls bridging engine start to the
            # first DMA-fed projection so the HAM clock-gate ramps to 8/8
            # before the dense work begins.  The chain writes the pj bank
            # (overwritten with start=True by the first projection).
            wup = ps_pj.tile([128, QBS], f32, tag="pj", name="warmup")
            prev_w = None
            for i in range(WARMUP_MMS):
                inst = mm(wup[:, 0:256], dummy[:, 0:128], dummy[:],
                          start=True, stop=True)
                if prev_w is not None:
                    add_dep_helper(inst.ins, prev_w.ins, sync=True,
                                   reason="warmup-chain")
                prev_w = inst

            # HAM bridge: unchained dummy matmuls into the (startup-idle)
            # ctx psum bank, interleaved between the DMA-gated projection
            # matmuls so the activity monitor never re-throttles during
            # the dribbled startup.  The ctx/v-proj allocations that
            # follow are WAR-ordered behind these writes automatically.
            bridge_ps = ps_cx.tile([128, 4 * (HD + 1)], f32, tag="cx",
                                   name="bridge")

            def bridge(n, dep=None):
                # dep-chained so the static scheduler places them right
                # after `dep` instead of hoisting all always-ready
                # bridges to the front
                for _ in range(n):
                    inst = mm(bridge_ps[:, 0:128], dummy[:, 0:128],
                              dummy[:, 0:128], start=True, stop=True)
                    if dep is not None:
                        add_dep_helper(inst.ins, dep.ins, sync=True,
                                       reason="ham-bridge")
                    dep = inst

            def warmer(dep):
                """One dummy matmul dep-chained on an exp: keeps the HAM
                activity window busy across the PE lulls of the thin
                (ACT-bound) late iterations, whose ~5us idle stretches
                otherwise re-throttle the PE clock to 1.2GHz."""
                ps = ps_pj.tile([128, QBS], f32, tag="pj", name="warmer")
                inst = mm(ps[:, 0:128], dummy[:, 0:128], dummy[:, 0:128],
                          start=True, stop=True)
                add_dep_helper(inst.ins, dep.ins, sync=True,
                               reason="ham-warmer")

            WARM_ITERS = {4, 5, 6, 7}

            # ---- work units ----
            def qk_proj_block(w_sb, bias_ap, dst, m, nb, split_k=False,
                              pool=None, bridges=0, half=None):
                # half=0/1: project only 256 of the block's 512 positions
                # -- a queued piece that fits the ~1.8us batch-gap budget
                if half is None:
                    c0, cw = nb * QBS, QBS
                else:
                    c0, cw = nb * QBS + half * 256, 256
                ns = slice(c0, c0 + cw)
                if pool is None:
                    ps = ps_pj.tile([128, QBS], f32, tag="pj", name="pspj")
                else:
                    # startup only: borrow a scores-pool buffer so the Q
                    # projection does not WAR-wait the K drain on ps_pj
                    ps_full = ps_sc.tile([128, 3 * QBS], f32, tag="sc",
                                         name="pspjq")
                    ps = ps_full[:, 0:QBS]
                mms = []
                for ft in range(FT):
                    mms.append(mm(
                        ps[:, 0:cw],
                        w_slice(w_sb, ft, DS, m * 128, (m + 1) * 128),
                        xt_ap(ft, c0, c0 + cw),
                        start=(ft == 0), stop=(ft == FT - 1)))
                    if bridges and ft < FT - 1:
                        # keep the HAM activity window busy across the
                        # DMA-arrival-paced accumulation chain
                        bridge(bridges, dep=mms[-1])
                if split_k:
                    nc.vector.tensor_scalar_add(
                        kt_pad[m][0][0:64, ns], ps[0:64, 0:cw],
                        bias_ap[0:64, :])
                    nc.vector.tensor_scalar_add(
                        kt_pad[m][1][64:128, ns], ps[64:128, 0:cw],
                        bias_ap[64:128, :])
                else:
                    nc.vector.tensor_scalar_add(dst[:, ns], ps[:, 0:cw],
                                                bias_ap)
                return mms

            def v_proj_block(st):
                # alternate psum between the pj and (pre-ctx idle) cx
                # banks: consecutive V-projections then overlap the DVE
                # drain of one with the matmuls of the next instead of
                # WAR-serializing on a single bank
                ws0 = st * 128
                if st % 2 == 0:
                    ps = ps_pj.tile([128, QBS], f32, tag="pj", name="pspjv")
                else:
                    ps = ps_cx.tile([128, 4 * (HD + 1)], f32, tag="cx",
                                    name="pscxv")
                for ft in range(FT):
                    mm(ps[:, 0:VW],
                       xt_ap(ft, ws0, ws0 + 128),
                       w_slice(wv_sb, ft, VW, 0, VW),
                       start=(ft == 0), stop=(ft == FT - 1))
                nc.vector.tensor_scalar_mul(
                    vones[st][:], ps[:, 0:VW], em_sb[:, st:st + 1])
                for hh in range(HPC):
                    c = hh * (HD + 1) + HD
                    nc.gpsimd.tensor_copy(
                        vones[st][:, c:c + 1], em_sb[:, st:st + 1])

            BATCHES = [(0, 3), (3, 3), (6, 3), (9, 3), (12, 3), (15, 1)]

            def scores_batch(hp, qb, eA, eB, k0, nk, half=None):
                """Scores for `nk` key-tiles of 512 queries.  Stationary =
                the head's zero-padded K^T tile; moving = the full 128-row
                q tile (the other head's rows hit the zero weights)."""
                qs = slice(qb * QBS, (qb + 1) * QBS)
                w = nk * QBS
                es = slice(k0 * QBS, k0 * QBS + w)
                psA = psB = None
                if half is None or half == 0:
                    psA = ps_sc.tile([128, 3 * QBS], f32, tag="sc",
                                     name="pscA")
                if half is None or half == 1:
                    psB = ps_sc.tile([128, 3 * QBS], f32, tag="sc",
                                     name="pscB")
                for j in range(nk):
                    kt = k0 + j
                    ks0 = kt * 128
                    js = slice(j * QBS, (j + 1) * QBS)
                    for (ps, sd) in ((psA, 0), (psB, 1)):
                        if ps is None:
                            continue
                        mm(ps[:, js],
                           kt_pad[hp][sd][:, ks0:ks0 + 128],
                           qt[hp][:, qs])
                out = []
                for (ps, e) in ((psA, eA), (psB, eB)):
                    if ps is None:
                        continue
                    out.append(nc.scalar.activation(
                        e[:, es], ps[:, 0:w], AF.Exp, bias=ebias[:],
                        scale=0.125))
                return out

            def ctx_pieces(prev_state, fine_tail=False):
                """The ctx work for iteration `prev_state`, split into 10
                independently-emittable pieces (per head: 4 q-subtile
                matmul groups + 1 eviction), queued between the next
                iteration's scores batches.  With fine_tail (the very
                last head), each group evicts its own q-subtile as soon
                as its accumulation stops and the output ships in two
                halves, collapsing the end-of-kernel serial chain."""
                hp, qb, eA, eB = prev_state
                state = {"ot": None, "cps": {}}

                def mk_mm_group(a, e, qq):
                    def f():
                        cpsb = state["cps"].get(a)
                        if cpsb is None:
                            cpsb = ps_cx.tile([128, 4 * (HD + 1)], f32,
                                              tag="cx", name="cps")
                            state["cps"][a] = cpsb
                        hh = 2 * hp + a
                        cps = cpsb[:, qq * (HD + 1):(qq + 1) * (HD + 1)]
                        for ktile in range(KT):
                            lo = ktile * QBS + qq * 128
                            mm(cps, e[:, lo:lo + 128],
                               vones[ktile][:,
                                            hh * (HD + 1):(hh + 1) * (HD + 1)],
                               start=(ktile == 0), stop=(ktile == KT - 1))
                    return f

                def mk_evict(a):
                    def f():
                        hh = 2 * hp + a
                        cpsb = state["cps"][a]
                        # per-head output tile + DMA: head A's output
                        # ships while head B's ctx is still accumulating
                        ot = opool.tile([128, 4 * HD], f32, tag="ot",
                                        bufs=4, name="ot")
                        r4 = rpool.tile([128, 4], f32, tag="r", bufs=2,
                                        name="r")
                        nc.vector.reciprocal(
                            r4[:], cpsb[:, HD:4 * (HD + 1):HD + 1])
                        for qq in range(4):
                            cps = cpsb[:, qq * (HD + 1):(qq + 1) * (HD + 1)]
                            nc.vector.scalar_tensor_tensor(
                                ot[:, qq * HD:(qq + 1) * HD],
                                cps[:, 0:HD], r4[:, qq:qq + 1],
                                bvb_sb[:, hh * HD:(hh + 1) * HD],
                                op0=OP.mult, op1=OP.add)
                        qt0 = qb * 4 * 128
                        dram = out_d.ap()[qt0:qt0 + 4 * 128,
                                          hh * HD:(hh + 1) * HD]
                        nc.sync.dma_start(
                            dram.rearrange("(qq p) c -> p qq c", qq=4),
                            ot[:])
                    return f

                def mk_fine(a, e, qq):
                    def f():
                        mk_mm_group(a, e, qq)()
                        hh = 2 * hp + a
                        cpsb = state["cps"][a]
                        if qq == 0:
                            state["ot"] = opool.tile([128, 4 * HD], f32,
                                                     tag="ot", bufs=4,
                                                     name="ot")
                        ot = state["ot"]
                        r1 = rpool.tile([128, 4], f32, tag="r", bufs=2,
                                        name="r1")
                        nc.vector.reciprocal(
                            r1[:, 0:1],
                            cpsb[:, qq * (HD + 1) + HD:
                                 qq * (HD + 1) + HD + 1])
                        cps = cpsb[:, qq * (HD + 1):(qq + 1) * (HD + 1)]
                        nc.vector.scalar_tensor_tensor(
                            ot[:, qq * HD:(qq + 1) * HD],
                            cps[:, 0:HD], r1[:, 0:1],
                            bvb_sb[:, hh * HD:(hh + 1) * HD],
                            op0=OP.mult, op1=OP.add)
                        if qq in (1, 3):
                            p = qq // 2
                            qt0 = qb * 4 * 128 + p * 256
                            dram = out_d.ap()[qt0:qt0 + 256,
                                              hh * HD:(hh + 1) * HD]
                            nc.sync.dma_start(
                                dram.rearrange("(qq p) c -> p qq c", qq=2),
                                ot[:, p * 2 * HD:(p + 1) * 2 * HD])
                    return f

                pieces = []
                for a, e in ((0, eA), (1, eB)):
                    if fine_tail and a == 1:
                        for qq in range(4):
                            pieces.append((0.6, mk_fine(a, e, qq)))
                    else:
                        for qq in range(4):
                            pieces.append((0.5, mk_mm_group(a, e, qq)))
                        pieces.append((0.25, mk_evict(a)))
                return pieces

            # ---- emission schedule ----
            # A single global work queue of (pe_cost_us, fn, deadline)
            # pieces, pumped between scores batches so the PE fills its
            # psum-WAR windows with useful work and ACT never starves at
            # an iteration boundary.  Tile resolves data deps in EMISSION
            # order, so a projection emitted after its consumer (or a ctx
            # piece emitted after its e-tile buffer is reallocated)
            # silently reads the wrong data -- every piece therefore
            # carries a deadline key it*16+bi ("must be emitted before
            # batch bi of iteration it"; bi=15 means end of iteration)
            # and drain_due() force-emits due pieces, FIFO, before each
            # batch.  Front-heavy work (V/Q/K projections) thus rolls
            # into the back half's PE slack as far as its deadline
            # allows: the per-iteration PE/ACT balance is ~131us PE busy
            # against a 134us ACT span, but iterations 0-3 hold nearly
            # all the projections while 4-7 have ~5us slack each.
            from collections import deque
            work_q = deque()

            def key(it, bi=15):
                return it * 16 + bi

            def pump(budget):
                while work_q and budget > 0:
                    cost, fn, _ = work_q.popleft()
                    fn()
                    budget -= cost

            def drain_due(k):
                # force-emit, in FIFO order, everything up to and
                # including the LAST due piece (due pieces may sit
                # behind later-deadline ones)
                due = None
                for idx, (_, _, d) in enumerate(work_q):
                    if d <= k:
                        due = idx
                if due is not None:
                    for _ in range(due + 1):
                        _, fn, _ = work_q.popleft()
                        fn()

            def k1_block(nb, h=None):
                qk_proj_block(wk_sb, bk_sb[1], None, 1, nb, split_k=True,
                              half=h)

            def q1_block(nb, h=None):
                qk_proj_block(wq_sb, bq_sb[1], qt[1], 1, nb, half=h)

            PROJ_COST = 1.9   # 8 N=512 matmuls + pj-psum drain wait
            VPROJ_COST = 1.0  # 8 N=260 matmuls, drain overlapped (pj/cx)
            # fillers[it] = (cost, fn, deadline) queued at iteration it,
            # ordered by deadline.  V-projections must precede ctx(0)'s
            # pieces (FIFO gives that); the m0 q-block projections feed
            # iterations 2/3; the m1 K/Q projections feed specific
            # batches of iteration 4 (kt_pad[1] keys nb*512.. are first
            # read by the batch covering them), so they may slide into
            # iteration 4 itself.
            fillers = {
                0: [(PROJ_COST, lambda: qk_proj_block(
                        wq_sb, bq_sb[0], qt[0], 0, 2), key(2, 0))]
                   + [(VPROJ_COST, (lambda st=st: v_proj_block(st)),
                       key(2, 15))
                      for st in range(6)],
                1: [(0.1, lambda: nc.vector.memset(
                        kt_pad[1][0][64:128, :], 0.0), key(2)),
                    (0.1, lambda: nc.vector.memset(
                        kt_pad[1][1][0:64, :], 0.0), key(2))]
                   + [(VPROJ_COST, (lambda st=st: v_proj_block(st)),
                       key(2, 15))
                      for st in range(6, 12)]
                   + [(PROJ_COST, lambda: qk_proj_block(
                          wq_sb, bq_sb[0], qt[0], 0, 3), key(3, 0))],
                2: [(VPROJ_COST, (lambda st=st: v_proj_block(st)),
                     key(2, 15))
                    for st in range(12, ST)],
                # m1 projections as 256-position HALF blocks (~0.95us
                # each, fits the batch-gap budget) with deadlines at the
                # exact batch that first reads each half's keys
                3: [(1.0, lambda: k1_block(0, 0), key(3, 2)),
                    (1.0, lambda: k1_block(0, 1), key(3, 3)),
                    (1.0, lambda: q1_block(0, 0), key(3, 4)),
                    (1.0, lambda: q1_block(0, 1), key(3, 5)),
                    (1.0, lambda: k1_block(1, 0), key(4, 1)),
                    (1.0, lambda: k1_block(1, 1), key(4, 2))],
                4: [(1.0, lambda: k1_block(2, 0), key(4, 2)),
                    (1.0, lambda: k1_block(2, 1), key(4, 3)),
                    (1.0, lambda: k1_block(3, 0), key(4, 4)),
                    (1.0, lambda: k1_block(3, 1), key(4, 4)),
                    (1.0, lambda: q1_block(1, 0), key(5, 0)),
                    (1.0, lambda: q1_block(1, 1), key(5, 0))],
                5: [(1.0, lambda: q1_block(2, 0), key(6, 0)),
                    (1.0, lambda: q1_block(2, 1), key(6, 0))],
                6: [(1.0, lambda: q1_block(3, 0), key(7, 0)),
                    (1.0, lambda: q1_block(3, 1), key(7, 0))],
            }

            def queue_ctx(state, j):
                # ctx(j)'s A-half must be emitted before iteration j+3
                # reallocates eA's buffer (bufs=3); eB has bufs=4, so the
                # B-half may lag one iteration further into the back
                # half's PE slack
                pieces = ctx_pieces(state)
                work_q.extend((c, f, key(j + 2)) for (c, f) in pieces[:5])
                work_q.extend((c, f, key(j + 3)) for (c, f) in pieces[5:])

            prev = None
            for it in range(8):
                hp, qb = divmod(it, QB)
                eA = epool.tile([128, KT * QBS], f16, tag="eA", bufs=3,
                                name="eA")
                eB = epool.tile([128, KT * QBS], f16, tag="eB", bufs=4,
                                name="eB")
                # queue this iteration's fillers, then pending ctx
                # pieces.  ctx(0) is queued at iteration 2, AFTER the
                # last V-projections (its matmul groups read all 16
                # vones; FIFO order guarantees the precedence); ctx(j>0)
                # is queued at iteration j+1.
                work_q.extend(fillers.get(it, []))
                if it == 2:
                    queue_ctx(state0, 0)
                if prev is not None and it >= 2:
                    queue_ctx(prev, it - 1)

                def batch(k0, nk, half=None):
                    es = scores_batch(hp, qb, eA, eB, k0, nk, half)
                    if it in WARM_ITERS:
                        for e_inst in es:
                            warmer(e_inst)

                if it == 0:
                    # m0 Q/K projections interleaved with iteration-0's
                    # scores batches, each batch emitted as soon as the K
                    # key-blocks it reads are projected; HAM bridge
                    # matmuls (dep-chained per projection matmul) fill
                    # the DMA-arrival-paced stretches so the PE
                    # clock-gate stays at 8/8 through the startup.
                    # each K-projection emitted one batch AHEAD of its
                    # first consumer: the psum-drain adds' PE waits
                    # resolve coarsely (they fire only after later
                    # scheduled matmuls), so the extra batch of slack
                    # keeps them off the exp critical path
                    k_mms = qk_proj_block(wk_sb, bk_sb[0], None, 0, 0,
                                          split_k=True, bridges=2)
                    qk_proj_block(wq_sb, bq_sb[0], qt[0], 0, 0, pool="sc")
                    deferred_dmas_nb1(k_mms[3])
                    batch(0, 3)
                    k1_mms = qk_proj_block(wk_sb, bk_sb[0], None, 0, 1,
                                           split_k=True)
                    deferred_dmas_rest(k1_mms[-1])
                    qk_proj_block(wk_sb, bk_sb[0], None, 0, 2, split_k=True,
                                  bridges=1)
                    batch(3, 3)
                    qk_proj_block(wk_sb, bk_sb[0], None, 0, 3, split_k=True,
                                  bridges=1)
                    batch(6, 3)
                    qk_proj_block(wq_sb, bq_sb[0], qt[0], 0, 1)
                    batch(9, 3)
                    batch(12, 3)
                    pump(PROJ_COST)
                    batch(15, 1)
                    pump(2 * VPROJ_COST)
                elif it == 7:
                    # last iteration: all A-half batches first (draining
                    # the remaining queue); A's full ctx+eviction runs
                    # during the first B exp's drain; B's ctx then
                    # accumulates BATCH-SYNCHRONOUSLY (each batch's
                    # key-tiles right after its exp), so after the final
                    # exp only 4 matmuls + the per-q-subtile eviction
                    # and two half-output DMAs remain.
                    # A-half gaps drain only ONE exp (~1.6us) each, so
                    # the pump is halved vs the two-exp iterations --
                    # over-pumping here delays the next A batch
                    for bi, (k0, nk) in enumerate(BATCHES):
                        drain_due(key(it, bi))
                        batch(k0, nk, half=0)
                        pump(0.9 if nk == 3 else 0.4)
                    # drain ALL queue leftovers (ctx(6) etc.) before any
                    # B-half ctx touches the shared cx psum bank -- a
                    # late ctx(6) piece emitted mid-bsync would clobber
                    # the in-progress B accumulation (bufs=1)
                    pump(1e9)
                    pieces7 = ctx_pieces((hp, qb, eA, eB))
                    a_pieces = pieces7[:5]

                    bst = {}

                    def bsync(k0, nk):
                        # partial ctx accumulation for head B over this
                        # batch's key-tiles, all 4 q-subtiles.  start=True
                        # clears the WHOLE bank's has_written bits, so it
                        # is issued exactly once (first matmul of the
                        # bank); each region's first write then lands on
                        # has_written=0 and overwrites, later ones
                        # accumulate -- interleaved per-region chains
                        # with per-region start flags corrupt each other.
                        if "cps" not in bst:
                            bst["cps"] = ps_cx.tile([128, 4 * (HD + 1)],
                                                    f32, tag="cx",
                                                    name="cpsB7")
                        cpsb = bst["cps"]
                        for qq in range(4):
                            cps = cpsb[:, qq * (HD + 1):(qq + 1) * (HD + 1)]
                            for j in range(nk):
                                kt = k0 + j
                                lo = kt * QBS + qq * 128
                                mm(cps, eB[:, lo:lo + 128],
                                   vones[kt][:, (2 * hp + 1) * (HD + 1):
                                             (2 * hp + 2) * (HD + 1)],
                                   start=(kt == 0 and qq == 0),
                                   stop=(kt == KT - 1))

                    # B-half gaps hold only ~1.6us of ACT drain each (one
                    # exp per batch), so A's 2.8us of ctx splits across
                    # the first two gaps and the first two bsyncs share
                    # gap 3
                    after_b = {
                        0: [a_pieces[0][1], a_pieces[1][1]],
                        1: [a_pieces[2][1], a_pieces[3][1], a_pieces[4][1]],
                        2: [lambda: bsync(0, 3), lambda: bsync(3, 3)],
                        3: [lambda: bsync(6, 3)],
                        4: [lambda: bsync(9, 3)],
                        5: [lambda: bsync(12, 3)],
                    }
                    for bi, (k0, nk) in enumerate(BATCHES):
                        batch(k0, nk, half=1)
                        for f in after_b[bi]:
                            f()
                    bsync(15, 1)
                    # B eviction: all 4 regions stop together in
                    # bsync(15,1), so one batched reciprocal + 4
                    # normalizes, output in two halves
                    cpsb = bst["cps"]
                    hh = 2 * hp + 1
                    otB = opool.tile([128, 4 * HD], f32, tag="ot", bufs=4,
                                     name="otB7")
                    r4 = rpool.tile([128, 4], f32, tag="r", bufs=2,
                                    name="r4B7")
                    nc.vector.reciprocal(
                        r4[:], cpsb[:, HD:4 * (HD + 1):HD + 1])
                    for qq in range(4):
                        cps = cpsb[:, qq * (HD + 1):(qq + 1) * (HD + 1)]
                        nc.vector.scalar_tensor_tensor(
                            otB[:, qq * HD:(qq + 1) * HD],
                            cps[:, 0:HD], r4[:, qq:qq + 1],
                            bvb_sb[:, hh * HD:(hh + 1) * HD],
                            op0=OP.mult, op1=OP.add)
                        if qq in (1, 3):
                            p = qq // 2
                            qt0 = qb * 4 * 128 + p * 256
                            dram = out_d.ap()[qt0:qt0 + 256,
                                              hh * HD:(hh + 1) * HD]
                            nc.sync.dma_start(
                                dram.rearrange("(qq p) c -> p qq c", qq=2),
                                otB[:, p * 2 * HD:(p + 1) * 2 * HD])
                    prev = None
                    continue
                else:
                    for bi, (k0, nk) in enumerate(BATCHES):
                        drain_due(key(it, bi))
                        batch(k0, nk)
                        pump(PUMP_BUDGET if nk == 3 else PUMP_BUDGET_SMALL)
                drain_due(key(it))
                if it == 0:
                    state0 = (hp, qb, eA, eB)
                prev = (hp, qb, eA, eB)

    return nc


def _get_program(split_waits=True):
    key = ("nc", split_waits)
    if key not in _CACHE:
        _CACHE[key] = _build_program(split_waits)
    return _CACHE[key]


def _make_in_maps(hidden_states, attention_mask, Wq, bq, Wk, bk, Wv, bv):
    hidden = np.ascontiguousarray(np.asarray(hidden_states, dtype=np.float32))
    mask = np.asarray(attention_mask, dtype=np.float32)
    Wq = np.asarray(Wq, dtype=np.float32)
    Wk = np.asarray(Wk, dtype=np.float32)
    Wv = np.asarray(Wv, dtype=np.float32)
    bq = np.asarray(bq, dtype=np.float32)
    bk = np.asarray(bk, dtype=np.float32)
    bv = np.asarray(bv, dtype=np.float32)

    WqT = Wq.T  # [in, out]
    WkT = Wk.T
    WvT = Wv.T

    def pack_ft(w):  # [H, C] -> [128, FT*C] with col block ft*C
        C = w.shape[1]
        out = np.empty((128, FT * C), np.float16)
        for ft in range(FT):
            out[:, ft * C:(ft + 1) * C] = w[ft * 128:(ft + 1) * 128, :]
        return np.ascontiguousarray(out)

    in_maps = []
    for c in range(NCORES):
        b, hg = divmod(c, HPC)
        cols = slice(hg * DS, (hg + 1) * DS)
        xT = np.ascontiguousarray(hidden[b].T.astype(np.float16))
        wq = pack_ft(WqT[:, cols].astype(np.float16))
        wk = pack_ft(WkT[:, cols].astype(np.float16))
        wv_base = WvT[:, cols]
        wvT = np.zeros((H, VW), np.float32)
        for hh in range(HPC):
            wvT[:, hh * (HD + 1):hh * (HD + 1) + HD] = \
                wv_base[:, hh * HD:(hh + 1) * HD]
        wv = pack_ft(wvT.astype(np.float16))
        bqk = np.empty((128, 4), np.float32)
        bqk[:, 0] = bq[cols][0:128]
        bqk[:, 1] = bq[cols][128:256]
        bqk[:, 2] = bk[cols][0:128]
        bqk[:, 3] = bk[cols][128:256]
        bvb = np.ascontiguousarray(np.tile(bv[cols][None, :], (128, 1)))
        em = np.ascontiguousarray(
            np.exp(mask[b, 0, 0, :]).reshape(KT, 128).T.astype(np.float32))
        in_maps.append({
            "xT": xT, "wq": wq, "wk": wk, "wv": wv,
            "bqk": np.ascontiguousarray(bqk), "bvb": bvb, "em": em,
        })
    return in_maps


def _assemble(results):
    out = np.empty((B, S, H), np.float32)
    for c in range(NCORES):
        b, hg = divmod(c, HPC)
        out[b][:, hg * DS:(hg + 1) * DS] = results[c]["out"]
    return out


def _run(in_maps, trace=False):
    from concourse.bass_utils import run_bass_kernel_spmd
    nc = _get_program()
    return run_bass_kernel_spmd(
        nc, in_maps, core_ids=list(range(NCORES)), trace=trace)


def kernel(**inputs):
    in_maps = _make_in_maps(**inputs)
    res = _run(in_maps, trace=False)
    return _assemble(res.results)
